# revision 51
# baseline (speedup 1.0000x reference)
"""NGCF-style GNN forward on 8 Trainium2 NeuronCores.

Strategy: host precomputes dense [4096,4096] message matrices (edge
multiplicity folded in) sharded column-wise per core (512 dest nodes
each) and pre-packed into SBUF arena layout; device runs the full layer
stack in bf16 (fp32 PSUM accumulation) with message-passing outputs
AllGathered between layers. The prediction layer is row-parallel: each
core computes scores for its own 512 nodes against all 41476 classes
(padded to 41984), written bf16; the host concatenates rows, upcasts,
and adds pred_b. GCN biases are skipped (they cancel inside BatchNorm).

Message matrices stream through two SBUF arenas (a_gcn is reused by
both GCN layers, m_gat by both GAT layers) as single whole-matrix DMAs.
"""
import sys
sys.path.insert(0, '/opt/trn_rl_repo')
import numpy as np
import ml_dtypes
from concourse import bass, tile, mybir
from concourse.bass_utils import run_bass_kernel_spmd
from concourse.vector_clock import ScopedClock
from concourse.tile_clock_wait import TileClockWait  # noqa: F401

AF = mybir.ActivationFunctionType
ALU = mybir.AluOpType
AX = mybir.AxisListType
FP32 = mybir.dt.float32
BF16 = mybir.dt.bfloat16
NPBF = ml_dtypes.bfloat16

N = 4096
NCORES = 8
CH = 512            # dest nodes per core (message-pass column shard)
NT = N // 128       # 32 node r-tiles
NCLS = 41476
NPAD = 41984        # 82 * 512
BN_EPS = 1e-5
RG = [list(range(NCORES))]
PRED_HILO = False  # bf16 hi+lo split of x8 (extra precision, ~40us slower)


# ---- workaround: this walrus build rejects instructions with >1 sync-wait;
# TileContext's final drain aggregates one wait per semaphore, so split them
# across single-wait SP nops.
def _patched_drain_and_barrier(self, tick_clock, wait_clock):
    nc = self.nc
    probe = nc.sync.nop(nofuse=True, hint="drain_wait_split").ins
    wait_clock.add_sem_waits(probe, ScopedClock({None: tick_clock.global_clock}))
    waits = list(probe.sync_info.on_wait) if probe.sync_info is not None else []
    if probe.sync_info is not None and len(waits) > 1:
        probe.sync_info = mybir.SyncInfo(on_wait=waits[:1], on_update=[])
        for w in waits[1:]:
            extra = nc.sync.nop(nofuse=True, hint="drain_wait_split").ins
            extra.sync_info = mybir.SyncInfo(on_wait=[w], on_update=[])
    nc.sync.drain()
    nc.all_engine_barrier()
    popped = nc._tile_sem_poison_stack.pop()
    assert popped is self._sem_poison
    nc.clear_and_free_semaphores(list(self.sems.allocated().values()))
    nc.all_engine_barrier()


tile.TileContext._drain_and_barrier = _patched_drain_and_barrier


# Same walrus limitation for mid-program instructions: during lowering,
# instructions are committed in final order, so extra waits can be peeled
# onto same-engine nops emitted just before the carrying instruction.
_orig_commit_and_lower = tile.TileContext._commit_and_lower


def _patched_commit_and_lower(self, inst, original_block, old_bb_map, bb_to_exit_bb):
    si = getattr(inst, "sync_info", None)
    eng_map = self.nc.engines
    if (si is not None and len(si.on_wait) > 1
            and type(inst).__module__.startswith("bass_rust")
            and inst.engine in eng_map):
        waits = list(si.on_wait)
        eng = eng_map[inst.engine]
        for w in waits[:-1]:
            nop_ins = eng.nop(nofuse=True, hint="wait_split").ins
            nop_ins.sync_info = mybir.SyncInfo(on_wait=[w], on_update=[])
        inst.sync_info = mybir.SyncInfo(on_wait=waits[-1:],
                                        on_update=list(si.on_update))
    return _orig_commit_and_lower(self, inst, original_block, old_bb_map,
                                  bb_to_exit_bb)


tile.TileContext._commit_and_lower = _patched_commit_and_lower


def _batch_norm(nc, bn_pool, mt, fscr, scr2, g_col, b_col, inv_n):
    """Per-partition BN stats over the free dim of mt [128, n] (bf16),
    shifted two-pass variance: var = E[(x-mu)^2] (avoids cancellation).
    Returns (s, bp) [128,1] fp32 APs so caller applies relu(s*x + bp)."""
    mu_raw = bn_pool.tile([128, 1], FP32, name="mu_raw", bufs=2)
    nc.vector.reduce_sum(mu_raw[:], mt, axis=AX.X)
    mu_neg = bn_pool.tile([128, 1], FP32, name="mu_neg", bufs=2)
    nc.vector.tensor_scalar_mul(mu_neg[:], mu_raw[:], -inv_n)
    nc.vector.tensor_scalar(fscr, mt, mu_neg[:], None, ALU.add)
    sumsq = bn_pool.tile([128, 1], FP32, name="sumsq", bufs=2)
    nc.vector.scalar_tensor_tensor(scr2, fscr, 1.0, fscr, ALU.bypass, ALU.mult,
                                   accum_out=sumsq[:])
    var = bn_pool.tile([128, 1], FP32, name="var", bufs=2)
    nc.vector.tensor_scalar(var[:], sumsq[:], inv_n, BN_EPS, ALU.mult, ALU.add)
    std = bn_pool.tile([128, 1], FP32, name="std", bufs=2)
    nc.scalar.activation(std[:], var[:], AF.Sqrt)
    rinv = bn_pool.tile([128, 1], FP32, name="rinv", bufs=2)
    nc.vector.reciprocal(rinv[:], std[:])
    s = bn_pool.tile([128, 1], FP32, name="s", bufs=2)
    nc.vector.tensor_tensor(s[:], g_col, rinv[:], ALU.mult)
    mu = bn_pool.tile([128, 1], FP32, name="mu", bufs=2)
    nc.vector.tensor_scalar_mul(mu[:], mu_raw[:], inv_n)
    sm = bn_pool.tile([128, 1], FP32, name="sm", bufs=2)
    nc.vector.tensor_tensor(sm[:], s[:], mu[:], ALU.mult)
    bp = bn_pool.tile([128, 1], FP32, name="bp", bufs=2)
    nc.vector.tensor_tensor(bp[:], b_col, sm[:], ALU.subtract)
    return s, bp


def build_program(no_collective=False, stream_a=False, no_pred=False, repeat=1,
                  debug_taps=False):
    nc = bass.Bass(num_devices=NCORES)

    def ein(name, shape, dt=BF16):
        return nc.dram_tensor(name, shape, dt, kind="ExternalInput")

    # all big operands pre-packed on host into SBUF arena layout
    d_xin = ein("x_inT", [128, N])
    d_w1 = ein("w1", [128, 1024])
    d_b1 = ein("b1", [128, 8], FP32)
    d_w2 = ein("w2", [128, 4096])          # packed [1024,512] -> 8 col blocks
    d_b2 = ein("b2", [128, 4], FP32)
    d_gw1 = ein("gcn_w1", [128, 1024])     # packed [512,256] -> 4 col blocks
    d_bn1g = ein("bn1_g", [128, 2], FP32)
    d_bn1b = ein("bn1_b", [128, 2], FP32)
    d_gw2 = ein("gcn_w2", [128, 256])      # packed [256,128] -> 2 col blocks
    d_bn2g = ein("bn2_g", [128, 1], FP32)
    d_bn2b = ein("bn2_b", [128, 1], FP32)
    d_swl = ein("sage_wl", [128, 128])
    d_sbl = ein("sage_bl", [128, 1], FP32)
    d_swr = ein("sage_wr", [128, 128])
    d_cw0 = ein("cheb_w0", [128, 128])
    d_cw1 = ein("cheb_w1", [128, 128])
    d_cb = ein("cheb_b", [128, 1], FP32)
    d_gwva1 = ein("gwva1", [128, 129])
    d_vd1 = ein("vd1", [128, 1])
    d_g1b = ein("g1b", [128, 1], FP32)
    d_gwva2 = ein("gwva2", [128, 129])
    d_vd2 = ein("vd2", [128, 1])
    d_g2b = ein("g2b", [128, 1], FP32)
    # message matrices packed [128, NT*512]: col 512*rt+c = A[128*rt+p, c]
    d_agcn = ein("a_gcn", [128, NT * CH])
    d_asage = ein("a_sage", [128, NT * CH])
    d_acheb = ein("a_cheb", [128, NT * CH])
    d_mgat = ein("m_gat", [128, NT * CH])
    d_pw = ein("pred_w", [128, NPAD])
    d_scores = nc.dram_tensor("scores", [CH, NPAD], BF16, kind="ExternalOutput")
    d_dbg = (nc.dram_tensor("dbg", [128, 4 * CH], BF16, kind="ExternalOutput")
             if debug_taps else None)
    d_dbgf = (nc.dram_tensor("dbgf", [128, CH], FP32, kind="ExternalOutput")
              if debug_taps else None)

    # collective bounce buffers (internal DRAM; outputs in shared space)
    cc_in = {}
    cc_out = {}
    for tag, rows in [("gcn1", 256), ("gcn2", 128), ("sage", 128),
                      ("cheb", 128), ("gat1", 128)]:
        cc_in[tag] = nc.dram_tensor(f"ccin_{tag}", [rows, CH], BF16)
        cc_out[tag] = nc.dram_tensor(f"ccout_{tag}", [NCORES * rows, CH], BF16,
                                     addr_space="Shared")

    def emit_pass():
        with (
            tc.tile_pool(name="wts", bufs=1) as wp,
            tc.tile_pool(name="big", bufs=1) as bp_,
            tc.tile_pool(name="amat", bufs=1) as amp,
            tc.tile_pool(name="aux", bufs=1) as ax,
            tc.tile_pool(name="bn", bufs=1) as bnp,
        ):
            # ---- persistent SBUF arenas (bf16)
            t_h2 = bp_.tile([128, 16384], BF16, name="t_h2")
            t_b2 = bp_.tile([128, 8192], BF16, name="t_b2")
            t_b3 = bp_.tile([128, 8192], BF16, name="t_b3")
            a_arena = amp.tile([128, 16384], BF16, name="a_arena")   # gcn / cheb
            b_arena = amp.tile([128, 16384], BF16, name="b_arena")   # sage / gat
            fscr = ax.tile([128, 4096], FP32, name="fscr")  # BN sumsq scratch
            cc0 = ax.tile([128, CH], BF16, name="cc0")
            cc1 = ax.tile([128, CH], BF16, name="cc1")
            cc0f = ax.tile([128, CH], FP32, name="cc0f")  # gat2 out (pred input)
            loc0 = ax.tile([128, CH], BF16, name="loc0")
            adb = ax.tile([128, CH], FP32, name="adb")
            a_s_sb = ax.tile([128, NT], FP32, name="a_s_sb")
            ad_row = ax.tile([1, CH], FP32, name="ad_row")
            rec_row = ax.tile([1, CH], FP32, name="rec_row")
            ones_row = ax.tile([1, 128], BF16, name="ones_row")
            ones_r32 = ax.tile([1, 128], FP32, name="ones_r32")
            ones_col = ax.tile([128, 1], BF16, name="ones_col")
            nc.vector.memset(ones_row[:], 1.0)
            nc.vector.memset(ones_r32[:], 1.0)
            nc.vector.memset(ones_col[:], 1.0)

            # ---- weight loads (single-DMA pre-packed blocks); MLP-critical
            # tensors first so the MLP can start immediately
            w1_sb = wp.tile([128, 1024], BF16, name="w1_sb")
            nc.sync.dma_start(w1_sb[:], d_w1[:])
            b1_sb = wp.tile([128, 8], FP32, name="b1_sb")
            nc.sync.dma_start(b1_sb[:], d_b1[:])
            x_inT = t_b3[:, 0:4096]
            nc.sync.dma_start(x_inT, d_xin[:])
            w2_sb = wp.tile([128, 4096], BF16, name="w2_sb")
            nc.sync.dma_start(w2_sb[:], d_w2[:])
            b2_sb = wp.tile([128, 4], FP32, name="b2_sb")
            nc.sync.dma_start(b2_sb[:], d_b2[:])
            gw1_sb = wp.tile([128, 1024], BF16, name="gw1_sb")
            nc.sync.dma_start(gw1_sb[:], d_gw1[:])
            gw2_sb = wp.tile([128, 256], BF16, name="gw2_sb")
            nc.sync.dma_start(gw2_sb[:], d_gw2[:])
            bn1g_sb = wp.tile([128, 2], FP32, name="bn1g_sb")
            nc.sync.dma_start(bn1g_sb[:], d_bn1g[:])
            bn1b_sb = wp.tile([128, 2], FP32, name="bn1b_sb")
            nc.sync.dma_start(bn1b_sb[:], d_bn1b[:])
            bn2g_sb = wp.tile([128, 1], FP32, name="bn2g_sb")
            nc.sync.dma_start(bn2g_sb[:], d_bn2g[:])
            bn2b_sb = wp.tile([128, 1], FP32, name="bn2b_sb")
            nc.sync.dma_start(bn2b_sb[:], d_bn2b[:])
            swl_sb = wp.tile([128, 128], BF16, name="swl_sb")
            nc.sync.dma_start(swl_sb[:], d_swl[:])
            swr_sb = wp.tile([128, 128], BF16, name="swr_sb")
            nc.sync.dma_start(swr_sb[:], d_swr[:])
            sbl_sb = wp.tile([128, 1], FP32, name="sbl_sb")
            nc.sync.dma_start(sbl_sb[:], d_sbl[:])
            cw0_sb = wp.tile([128, 128], BF16, name="cw0_sb")
            nc.sync.dma_start(cw0_sb[:], d_cw0[:])
            cw1_sb = wp.tile([128, 128], BF16, name="cw1_sb")
            nc.sync.dma_start(cw1_sb[:], d_cw1[:])
            cb_sb = wp.tile([128, 1], FP32, name="cb_sb")
            nc.sync.dma_start(cb_sb[:], d_cb[:])
            gwva1_sb = wp.tile([128, 129], BF16, name="gwva1_sb")
            nc.sync.dma_start(gwva1_sb[:], d_gwva1[:])
            vd1_sb = wp.tile([128, 1], BF16, name="vd1_sb")
            nc.sync.dma_start(vd1_sb[:], d_vd1[:])
            g1b_sb = wp.tile([128, 1], FP32, name="g1b_sb")
            nc.sync.dma_start(g1b_sb[:], d_g1b[:])
            gwva2_sb = wp.tile([128, 129], BF16, name="gwva2_sb")
            nc.sync.dma_start(gwva2_sb[:], d_gwva2[:])
            vd2_sb = wp.tile([128, 1], BF16, name="vd2_sb")
            nc.sync.dma_start(vd2_sb[:], d_vd2[:])
            g2b_sb = wp.tile([128, 1], FP32, name="g2b_sb")
            nc.sync.dma_start(g2b_sb[:], d_g2b[:])

            # prefetch whole message matrices
            def load_arena(dst, src):
                if stream_a:
                    for q in range(8):
                        nc.sync.dma_start(dst[:, 2048 * q:2048 * (q + 1)],
                                          src[:, 2048 * q:2048 * (q + 1)])
                else:
                    nc.sync.dma_start(dst[:], src[:])

            load_arena(a_arena, d_agcn)     # gcn1+gcn2
            load_arena(b_arena, d_asage)    # sage

            def all_gather(tag):
                if no_collective:
                    return
                nc.gpsimd.collective_compute(
                    "AllGather", ALU.bypass, replica_groups=RG,
                    ins=[cc_in[tag][:].opt()], outs=[cc_out[tag][:].opt()])

            def gather_in(dst_fn, tag, rows=128, dst_ap=None):
                """DMA gathered rows back to SBUF (single rearranged DMA);
                under no_collective, replicate the local chunk so values
                stay bounded."""
                if no_collective or dst_ap is None or rows != 128:
                    for k in range(NCORES):
                        for h in range(rows // 128):
                            src = (cc_in[tag][128 * h:128 * (h + 1), :]
                                   if no_collective else
                                   cc_out[tag][rows * k + 128 * h:rows * k + 128 * (h + 1), :])
                            nc.sync.dma_start(dst_fn(k, h), src)
                    return
                src = cc_out[tag][:].rearrange("(k p) c -> p k c", p=128)
                dst = dst_ap.rearrange("p (k c) -> p k c", k=NCORES)
                nc.sync.dma_start(dst, src)

            # ============ MLP: x_inT -> h2T (T layout, [512f, 4096n]) ========
            # node chunks processed in pairs so each bias-carrying relu spans
            # 1024 cols (two psum banks); h1 pair layout: [t][j0 512 | j1 512]
            with tc.tile_pool(name="mlp_ps", bufs=2, space="PSUM") as mp:
                for jp in range(4):
                    for t in range(8):
                        ps1 = mp.tile([128, 1024], FP32, name="ps1", bufs=2)
                        for h in range(2):
                            nc.tensor.matmul(
                                ps1[:, 512 * h:512 * (h + 1)],
                                w1_sb[:, 128 * t:128 * (t + 1)],
                                x_inT[:, 1024 * jp + 512 * h:1024 * jp + 512 * (h + 1)],
                                start=True, stop=True)
                        nc.vector.tensor_scalar(
                            t_b2[:, 1024 * t:1024 * (t + 1)],
                            ps1[:], b1_sb[:, t:t + 1], 0.0, ALU.add, ALU.max)
                    for f2 in range(4):
                        ps2 = mp.tile([128, 1024], FP32, name="ps2", bufs=2)
                        for h in range(2):
                            for k in range(8):
                                nc.tensor.matmul(
                                    ps2[:, 512 * h:512 * (h + 1)],
                                    w2_sb[:, 512 * k + 128 * f2:512 * k + 128 * f2 + 128],
                                    t_b2[:, 1024 * k + 512 * h:1024 * k + 512 * (h + 1)],
                                    start=(k == 0), stop=(k == 7))
                        nc.scalar.activation(
                            t_h2[:, 4096 * f2 + 1024 * jp:4096 * f2 + 1024 * (jp + 1)],
                            ps2[:], AF.Relu, bias=b2_sb[:, f2:f2 + 1])

            # ============ GCN1 feature: h_g1 [n,256] in t_b2 ================
            with tc.tile_pool(name="g1f_ps", bufs=2, space="PSUM") as gp:
                for rt in range(NT):
                    psg = gp.tile([128, 256], FP32, name="psg", bufs=2)
                    for k in range(4):
                        nc.tensor.matmul(
                            psg[:], t_h2[:, 4096 * k + 128 * rt:4096 * k + 128 * rt + 128],
                            gw1_sb[:, 256 * k:256 * (k + 1)],
                            start=(k == 0), stop=(k == 3))
                    nc.vector.tensor_copy(t_b2[:, 256 * rt:256 * (rt + 1)], psg[:])

            # ============ GCN1 message (local chunk) + AllGather ============
            with tc.tile_pool(name="g1m_ps", bufs=1, space="PSUM") as gp:
                acc0 = gp.tile([128, 512], FP32, name="acc0")
                acc1 = gp.tile([128, 512], FP32, name="acc1")
                for rt in range(NT):
                    a_t = a_arena[:, 512 * rt:512 * (rt + 1)]
                    nc.tensor.matmul(acc0[:], t_b2[:, 256 * rt:256 * rt + 128], a_t,
                                     start=(rt == 0), stop=(rt == NT - 1))
                    nc.tensor.matmul(acc1[:], t_b2[:, 256 * rt + 128:256 * rt + 256],
                                     a_t, start=(rt == 0), stop=(rt == NT - 1))
                nc.vector.tensor_copy(cc0[:], acc0[:])
                nc.vector.tensor_copy(cc1[:], acc1[:])
            nc.sync.dma_start(cc_in["gcn1"][0:128, :], cc0[:])
            nc.sync.dma_start(cc_in["gcn1"][128:256, :], cc1[:])
            all_gather("gcn1")
            gather_in(lambda k, h: t_b3[:, 4096 * h + 512 * k:4096 * h + 512 * (k + 1)],
                      "gcn1", rows=256, dst_ap=t_b3[:, 0:8192])

            # ============ BN1 + relu -> x3T (t_h2 blocks 1,2) ===============
            bn_scr2 = t_h2[:, 12288:16384]  # h2 dead by BN time
            for t in range(2):
                mt = t_b3[:, 4096 * t:4096 * (t + 1)]
                s, bpc = _batch_norm(nc, bnp, mt, fscr[:], bn_scr2,
                                     bn1g_sb[:, t:t + 1], bn1b_sb[:, t:t + 1],
                                     1.0 / N)
                nc.scalar.activation(t_h2[:, 4096 * (1 + t):4096 * (2 + t)], mt,
                                     AF.Relu, bias=bpc[:], scale=s[:])

            # ============ GCN2 feature: h_g2 [n,128] in t_b2 ================
            with tc.tile_pool(name="g2f_ps", bufs=2, space="PSUM") as gp:
                for rt in range(NT):
                    psg = gp.tile([128, 128], FP32, name="psg2", bufs=2)
                    for k in range(2):
                        nc.tensor.matmul(
                            psg[:],
                            t_h2[:, 4096 * (1 + k) + 128 * rt:4096 * (1 + k) + 128 * rt + 128],
                            gw2_sb[:, 128 * k:128 * (k + 1)],
                            start=(k == 0), stop=(k == 1))
                    nc.vector.tensor_copy(t_b2[:, 128 * rt:128 * (rt + 1)], psg[:])

            # ============ GCN2 message + AllGather ==========================
            with tc.tile_pool(name="g2m_ps", bufs=1, space="PSUM") as gp:
                accm = gp.tile([128, 512], FP32, name="accm")
                for rt in range(NT):
                    nc.tensor.matmul(accm[:], t_b2[:, 128 * rt:128 * (rt + 1)],
                                     a_arena[:, 512 * rt:512 * (rt + 1)],
                                     start=(rt == 0), stop=(rt == NT - 1))
                nc.vector.tensor_copy(cc0[:], accm[:])
            nc.sync.dma_start(cc_in["gcn2"][:], cc0[:])
            # a_arena free after GCN2 message -> prefetch a_cheb into it
            load_arena(a_arena, d_acheb)
            all_gather("gcn2")
            gather_in(lambda k, h: t_b3[:, 512 * k:512 * (k + 1)], "gcn2",
                      dst_ap=t_b3[:, 0:4096])

            # ============ BN2 + relu -> x4T (t_b3 block 1) + local ==========
            mt_a = t_b3[:, 0:4096]
            s2, bp2 = _batch_norm(nc, bnp, mt_a, fscr[:], bn_scr2,
                                  bn2g_sb[:, 0:1], bn2b_sb[:, 0:1], 1.0 / N)
            x4T = t_b3[:, 4096:8192]
            nc.scalar.activation(x4T, mt_a, AF.Relu, bias=bp2[:], scale=s2[:])
            nc.scalar.activation(loc0[:], cc0[:], AF.Relu, bias=bp2[:], scale=s2[:])
            if debug_taps:
                nc.sync.dma_start(d_dbg[:, 0:512], loc0[:])

            # ============ SAGE ==============================================
            with tc.tile_pool(name="sage_ps", bufs=1, space="PSUM") as gp:
                for rt in range(NT):
                    psz = gp.tile([128, 128], FP32, name="psz", bufs=2)
                    nc.tensor.matmul(psz[:], x4T[:, 128 * rt:128 * (rt + 1)],
                                     swl_sb[:], start=True, stop=True)
                    nc.vector.tensor_copy(t_b2[:, 128 * rt:128 * (rt + 1)], psz[:])
                accs = gp.tile([128, 512], FP32, name="accs")
                for rt in range(NT):
                    nc.tensor.matmul(accs[:], t_b2[:, 128 * rt:128 * (rt + 1)],
                                     b_arena[:, 512 * rt:512 * (rt + 1)],
                                     start=(rt == 0), stop=False)
                nc.tensor.matmul(accs[:], swr_sb[:], loc0[:], start=False, stop=True)
                nc.scalar.activation(cc1[:], accs[:], AF.Relu, bias=sbl_sb[:])
            nc.sync.dma_start(cc_in["sage"][:], cc1[:])
            if debug_taps:
                nc.sync.dma_start(d_dbg[:, 512:1024], cc1[:])
            # b_arena free after SAGE message -> prefetch m_gat into it
            load_arena(b_arena, d_mgat)
            all_gather("sage")
            x5T = t_h2[:, 0:4096]
            gather_in(lambda k, h: x5T[:, 512 * k:512 * (k + 1)], "sage",
                      dst_ap=x5T)

            # ============ Cheb ==============================================
            with tc.tile_pool(name="cheb_ps", bufs=1, space="PSUM") as gp:
                for rt in range(NT):
                    psz = gp.tile([128, 128], FP32, name="psz1", bufs=2)
                    nc.tensor.matmul(psz[:], x5T[:, 128 * rt:128 * (rt + 1)],
                                     cw1_sb[:], start=True, stop=True)
                    nc.vector.tensor_copy(t_b2[:, 4096 + 128 * rt:4096 + 128 * (rt + 1)],
                                          psz[:])
                accc = gp.tile([128, 512], FP32, name="accc")
                for rt in range(NT):
                    nc.tensor.matmul(accc[:], t_b2[:, 4096 + 128 * rt:4096 + 128 * (rt + 1)],
                                     a_arena[:, 512 * rt:512 * (rt + 1)],
                                     start=(rt == 0), stop=False)
                nc.tensor.matmul(accc[:], cw0_sb[:], cc1[:], start=False, stop=True)
                nc.scalar.activation(cc0[:], accc[:], AF.Relu, bias=cb_sb[:])
            nc.sync.dma_start(cc_in["cheb"][:], cc0[:])
            if debug_taps:
                nc.sync.dma_start(d_dbg[:, 1024:1536], cc0[:])
            all_gather("cheb")
            x6T = t_b3[:, 0:4096]
            gather_in(lambda k, h: x6T[:, 512 * k:512 * (k + 1)], "cheb",
                      dst_ap=x6T)

            # ============ GAT layers ========================================
            def gat_layer(xT, xloc, gwva_sb, vd_sb, gb_sb, h_base, out_loc, tag):
                with tc.tile_pool(name=f"{tag}_ps", bufs=1, space="PSUM") as gp:
                    for rt in range(NT):
                        psh = gp.tile([128, 129], FP32, name="psh", bufs=2)
                        nc.tensor.matmul(psh[:], xT[:, 128 * rt:128 * (rt + 1)],
                                         gwva_sb[:], start=True, stop=True)
                        nc.vector.tensor_copy(
                            t_b2[:, h_base + 128 * rt:h_base + 128 * (rt + 1)],
                            psh[:, 0:128])
                        nc.vector.tensor_copy(a_s_sb[:, rt:rt + 1], psh[:, 128:129])
                    psd = gp.tile([1, 512], FP32, name="psd")
                    nc.tensor.matmul(psd[:], vd_sb[:], xloc[:], start=True, stop=True)
                    nc.vector.tensor_copy(ad_row[:], psd[:])
                    psb = gp.tile([128, 512], FP32, name="psb")
                    nc.tensor.matmul(psb[:], ones_r32[:], ad_row[:],
                                     start=True, stop=True)
                    nc.vector.tensor_copy(adb[:], psb[:])
                    accn = gp.tile([128, 512], FP32, name="accn")
                    accd = gp.tile([1, 512], FP32, name="accd")
                    for rp in range(NT // 2):
                        e_t = ax.tile([128, 1024], BF16, name="gat_et", bufs=2)
                        for h in range(2):
                            a_col = a_s_sb[:, 2 * rp + h:2 * rp + h + 1]
                            dst = e_t[:, 512 * h:512 * (h + 1)]
                            if h == 0:
                                nc.scalar.activation(dst, adb[:], AF.Lrelu,
                                                     bias=a_col, alpha=0.2)
                            else:
                                # lrelu on DVE: e0 = adb + a_s; max(e0, 0.2*e0)
                                e0 = ax.tile([128, 512], BF16, name="gat_e0",
                                             bufs=2)
                                nc.vector.tensor_scalar(e0[:], adb[:], a_col,
                                                        None, ALU.add)
                                nc.vector.scalar_tensor_tensor(
                                    dst, e0[:], 0.2, e0[:], ALU.mult, ALU.max)
                        x_t = ax.tile([128, 1024], BF16, name="gat_xt", bufs=2)
                        nc.scalar.activation(x_t[:], e_t[:], AF.Exp)
                        ab_t = ax.tile([128, 1024], BF16, name="gat_ab", bufs=2)
                        nc.vector.tensor_tensor(ab_t[:], x_t[:],
                                                b_arena[:, 1024 * rp:1024 * (rp + 1)],
                                                ALU.mult)
                        for h in range(2):
                            rt = 2 * rp + h
                            nc.tensor.matmul(accn[:],
                                             t_b2[:, h_base + 128 * rt:h_base + 128 * (rt + 1)],
                                             ab_t[:, 512 * h:512 * (h + 1)],
                                             start=(rt == 0), stop=(rt == NT - 1))
                            nc.tensor.matmul(accd[:], ones_col[:],
                                             ab_t[:, 512 * h:512 * (h + 1)],
                                             start=(rt == 0), stop=(rt == NT - 1))
                    nc.vector.reciprocal(rec_row[:], accd[:])
                    psr = gp.tile([128, 512], FP32, name="psr")
                    nc.tensor.matmul(psr[:], ones_r32[:], rec_row[:],
                                     start=True, stop=True)
                    nc.vector.tensor_copy(fscr[:, 0:512], accn[:])
                    prod = ax.tile([128, 512], FP32, name="gat_pr", bufs=2)
                    nc.vector.tensor_tensor(prod[:], fscr[:, 0:512], psr[:],
                                            ALU.mult)
                    r_t = ax.tile([128, 512], FP32, name="gat_rt", bufs=2)
                    nc.scalar.activation(r_t[:], prod[:], AF.Relu, bias=gb_sb[:])
                    m_n = ax.tile([128, 512], FP32, name="gat_mn", bufs=2)
                    nc.vector.tensor_scalar(m_n[:], prod[:], gb_sb[:], 0.0,
                                            ALU.add, ALU.min)
                    e2 = ax.tile([128, 512], FP32, name="gat_e2", bufs=2)
                    nc.scalar.activation(e2[:], m_n[:], AF.Exp)
                    nc.vector.scalar_tensor_tensor(out_loc[:], e2[:], -1.0, r_t[:],
                                                   ALU.add, ALU.add)

            gat_layer(x6T, cc0, gwva1_sb, vd1_sb, g1b_sb, 0, cc1, "gat1")
            nc.sync.dma_start(cc_in["gat1"][:], cc1[:])
            if debug_taps:
                nc.sync.dma_start(d_dbg[:, 1536:2048], cc1[:])
            all_gather("gat1")
            x7T = t_h2[:, 4096:8192]
            gather_in(lambda k, h: x7T[:, 512 * k:512 * (k + 1)], "gat1",
                      dst_ap=x7T)

            # GAT2: output only needed for local nodes (row-parallel pred)
            gat_layer(x7T, cc1, gwva2_sb, vd2_sb, g2b_sb, 4096, cc0f, "gat2")

            # ============ pred: scores[512 local nodes, NPAD] ===============
            # x8_loc = cc0f [128f, 512n] fp32, split into bf16 hi+lo so the
            # matmul keeps full input precision at bf16 rate (2 matmuls per
            # chunk, fp32 PSUM accumulate). pred_w bf16 streamed through fscr
            # (bitcast view); scores staged bf16 in t_h2 quarters.
            if no_pred:
                nc.sync.dma_start(d_scores[0:128, 0:512], cc0[:])
                return
            x8h = ax.tile([128, CH], BF16, name="x8h")
            x8l = ax.tile([128, CH], BF16, name="x8l")
            nc.vector.tensor_copy(x8h[:], cc0f[:])
            if PRED_HILO:
                nc.vector.scalar_tensor_tensor(x8l[:], cc0f[:], 1.0, x8h[:],
                                               ALU.bypass, ALU.subtract)
            cp_engines = [nc.vector, nc.scalar]
            NBLK = NPAD // 4096  # 10 full blocks
            REM = NPAD - NBLK * 4096  # 1024
            with tc.tile_pool(name="pred_ps", bufs=4, space="PSUM") as pp:
                i = 0
                for blk in range(NBLK + 1):
                    c0 = 4096 * blk
                    bw = 4096 if blk < NBLK else REM
                    pb = 4096 * (blk % 2)  # bf16 cols in t_b2 (gat h dead)
                    pwb = t_b2[:, pb:pb + bw]
                    nc.sync.dma_start(pwb, d_pw[:, c0:c0 + bw])
                    for nt in range(4):
                        osb = t_h2[:, 4096 * nt:4096 * nt + bw]
                        for cc in range(bw // 512):
                            rhs = t_b2[:, pb + 512 * cc:pb + 512 * (cc + 1)]
                            psp = pp.tile([128, 512], FP32, name="psp", bufs=4)
                            nc.tensor.matmul(psp[:], x8h[:, 128 * nt:128 * (nt + 1)],
                                             rhs, start=True, stop=not PRED_HILO)
                            if PRED_HILO:
                                nc.tensor.matmul(psp[:],
                                                 x8l[:, 128 * nt:128 * (nt + 1)],
                                                 rhs, start=False, stop=True)
                            eng = cp_engines[i % 2]
                            dst = osb[:, 512 * cc:512 * (cc + 1)]
                            if eng is nc.scalar:
                                eng.copy(dst, psp[:])
                            else:
                                eng.tensor_copy(dst, psp[:])
                            i += 1
                        nc.sync.dma_start(
                            d_scores[128 * nt:128 * (nt + 1), c0:c0 + bw], osb)

    with tile.TileContext(nc) as tc:
        for _rep in range(repeat):
            emit_pass()
    return nc


_PROG = None


def _get_program():
    global _PROG
    if _PROG is None:
        _PROG = build_program()
    return _PROG


def _pack(a):
    """[NT*128, CH] -> [128, NT*CH] arena layout (col 512*rt+c = A[128rt+p, c])."""
    return np.ascontiguousarray(
        a.reshape(NT, 128, CH).transpose(1, 0, 2).reshape(128, NT * CH))


def host_prep(inputs):
    bf = lambda a: np.ascontiguousarray(np.asarray(a, dtype=np.float32)).astype(NPBF)
    f32 = lambda a: np.ascontiguousarray(np.asarray(a), dtype=np.float32)
    ei = np.asarray(inputs["edge_index"])
    nx = np.asarray(inputs["node_x"])
    r = ei[0].astype(np.int64)
    c = ei[1].astype(np.int64)
    mult = np.bincount(r * N + c, minlength=N * N).reshape(N, N).astype(np.float32)

    deg = np.bincount(c, minlength=N).astype(np.float32) + 1.0
    dinv = deg ** -0.5
    a_gcn = mult * np.outer(dinv, dinv)
    idx = np.arange(N)
    a_gcn[idx, idx] += dinv * dinv

    cnt = np.bincount(c, minlength=N).astype(np.float32)
    a_sage = mult / np.maximum(cnt, 1.0)[None, :]

    deg0 = np.bincount(r, minlength=N).astype(np.float32)
    dinv0 = np.where(deg0 > 0, deg0 ** -0.5, 0.0).astype(np.float32)
    a_cheb = -(mult * np.outer(dinv0, dinv0))

    m_gat = mult
    m_gat[idx, idx] += 1.0

    ue = np.asarray(inputs["user_emb_w"])
    ie = np.asarray(inputs["item_emb_w"])
    x_in = np.concatenate([ue[nx[:, 0]], ie[nx[:, 1]]], axis=1)
    x_inT = bf(x_in.T)

    g1w = np.asarray(inputs["gat1_w"], dtype=np.float32)
    g2w = np.asarray(inputs["gat2_w"], dtype=np.float32)
    va1 = (g1w @ np.asarray(inputs["gat1_asrc"], dtype=np.float32)).reshape(128, 1)
    vd1 = (g1w @ np.asarray(inputs["gat1_adst"], dtype=np.float32)).reshape(128, 1)
    va2 = (g2w @ np.asarray(inputs["gat2_asrc"], dtype=np.float32)).reshape(128, 1)
    vd2 = (g2w @ np.asarray(inputs["gat2_adst"], dtype=np.float32)).reshape(128, 1)
    gwva1 = bf(np.concatenate([g1w, va1], axis=1))
    gwva2 = bf(np.concatenate([g2w, va2], axis=1))

    pw_pad = np.zeros((128, NPAD), dtype=np.float32)
    pw_pad[:, :NCLS] = np.asarray(inputs["pred_w"], dtype=np.float32)

    w2 = np.asarray(inputs["mlp_w2"], dtype=np.float32)  # [1024, 512]
    w2_pk = w2.reshape(8, 128, 512).transpose(1, 0, 2).reshape(128, 4096)
    gw1 = np.asarray(inputs["gcn_w1"], dtype=np.float32)  # [512, 256]
    gw1_pk = gw1.reshape(4, 128, 256).transpose(1, 0, 2).reshape(128, 1024)
    gw2 = np.asarray(inputs["gcn_w2"], dtype=np.float32)  # [256, 128]
    gw2_pk = gw2.reshape(2, 128, 128).transpose(1, 0, 2).reshape(128, 256)

    common = {
        "x_inT": x_inT,
        "w1": bf(inputs["mlp_w1"]),
        "b1": f32(np.asarray(inputs["mlp_b1"]).reshape(8, 128).T),
        "w2": bf(w2_pk),
        "b2": f32(np.asarray(inputs["mlp_b2"]).reshape(4, 128).T),
        "gcn_w1": bf(gw1_pk),
        "bn1_g": f32(np.asarray(inputs["bn1_g"]).reshape(2, 128).T),
        "bn1_b": f32(np.asarray(inputs["bn1_b"]).reshape(2, 128).T),
        "gcn_w2": bf(gw2_pk),
        "bn2_g": f32(np.asarray(inputs["bn2_g"]).reshape(128, 1)),
        "bn2_b": f32(np.asarray(inputs["bn2_b"]).reshape(128, 1)),
        "sage_wl": bf(inputs["sage_wl"]),
        "sage_bl": f32(np.asarray(inputs["sage_bl"]).reshape(128, 1)),
        "sage_wr": bf(inputs["sage_wr"]),
        "cheb_w0": bf(inputs["cheb_w0"]),
        "cheb_w1": bf(inputs["cheb_w1"]),
        "cheb_b": f32(np.asarray(inputs["cheb_b"]).reshape(128, 1)),
        "gwva1": gwva1, "vd1": bf(vd1),
        "g1b": f32(np.asarray(inputs["gat1_b"]).reshape(128, 1)),
        "gwva2": gwva2, "vd2": bf(vd2),
        "g2b": f32(np.asarray(inputs["gat2_b"]).reshape(128, 1)),
        "pred_w": bf(pw_pad),
    }
    in_maps = []
    for k in range(NCORES):
        sl = slice(CH * k, CH * (k + 1))
        m = dict(common)
        m["a_gcn"] = _pack(a_gcn[:, sl]).astype(NPBF)
        m["a_sage"] = _pack(a_sage[:, sl]).astype(NPBF)
        m["a_cheb"] = _pack(a_cheb[:, sl]).astype(NPBF)
        m["m_gat"] = _pack(m_gat[:, sl]).astype(NPBF)
        in_maps.append(m)
    return in_maps


def kernel(**inputs):
    in_maps = host_prep(inputs)
    nc = _get_program()
    res = run_bass_kernel_spmd(nc, in_maps, list(range(NCORES)))
    out = np.concatenate([np.asarray(res.results[k]["scores"])
                          for k in range(NCORES)], axis=0)[:, :NCLS]
    out = out.astype(np.float32) + np.asarray(inputs["pred_b"],
                                              dtype=np.float32)[None, :]
    return np.ascontiguousarray(out, dtype=np.float32)
